# revision 14
# baseline (speedup 1.0000x reference)
"""Trainium2 Bass kernel for nn_NodeModel (GNN message passing).

Reference computation:
    agg = segment_sum(edge_attr, edge_index[1], num_segments=N)     # scatter-add
    h   = relu(concat([x, agg, u[batch]], 1) @ W1 + b1)
    out = h @ W2 + b2 + x

Strategy (8 NeuronCores, graph-parallel by destination node):
  - Nodes are sorted by in-degree (descending) and dealt round-robin across
    the 8 cores, so every core sees the same degree profile. Per core the
    12544 owned nodes form the columns of all on-chip tensors.
  - The scatter-add runs as slab adds: the host lays the k-th incoming
    edge row of every node out as "pass k" (a [128, n_k] bf16 slab,
    n_k = #nodes with degree > k; nodes sorted by degree make every pass
    a dense prefix). Pass 0 is DMAed straight into agg; passes 1.. are
    DMAed to a staging tile (HWDGE, full rate) and added into agg by DVE
    tensor_add ops (~0.5 cyc/element bf16). No per-edge compute anywhere.
  - agg is chunked into 8 column ranges with independent add chains so
    chunks pipeline; the MLP consumes chunks as their chains complete.
  - MLP: h1T[h,n] accumulated in PSUM from W1x.T xT + W1a.T aggT + W1u.T ugT;
    ReLU+bias on ScalarE during evacuation; layer 2 in transposed
    orientation with the residual (+x) as an identity matmul and the bias
    as a rank-1 matmul. Output stays transposed [d, n] bf16 on device; the
    host de-transposes and un-permutes.
  - Everything is bf16 on the wire (rel err ~5e-3 incl. bf16 accumulation).
"""

from contextlib import ExitStack

import ml_dtypes
import numpy as np

N_NODES = 100000
N_EDGES = 1600000
D = 128          # node / edge feature dim
DG = 16          # global feature dim
H = 256          # hidden dim
G = 64           # graphs
NCORES = 8

NPC = 12544      # nodes per core
N_PAD = NCORES * NPC
CW = 1568        # agg chunk width (8 chunks per core)
NCHUNK = NPC // CW
NB = 392         # MLP group columns (CW % NB == 0)
MIN_PW = 64      # minimum pass width
PW_ALIGN = 1     # pass width alignment

BF16 = ml_dtypes.bfloat16

_PROFILE_RESULTS = [None]  # stash for test harness introspection


def _plan_passes(deg, order_nodes):
    """Shared-across-cores pass widths and per-chunk DRAM layout."""
    degmat = deg[order_nodes].reshape(NPC, NCORES)      # [pos, core]
    kmax = int(degmat.max())
    ks = np.arange(1, kmax)
    # count per core of nodes with degree > k  -> max over cores
    counts = (degmat[:, :, None] > ks[None, None, :]).sum(axis=0)  # [core, k-1]
    wk = counts.max(axis=0)

    widths = [NPC]
    for k in range(1, kmax):
        w = max(int(wk[k - 1]), MIN_PW)
        w = min(-(-w // PW_ALIGN) * PW_ALIGN, NPC)
        widths.append(w)

    seg_off = np.full((max(kmax, 1), NCHUNK), -1, dtype=np.int64)
    chunk_meta = []
    base = 0
    for c in range(NCHUNK):
        lo, hi = c * CW, (c + 1) * CW
        fulls = [k for k in range(1, kmax) if widths[k] >= hi]
        partials = []
        seg_off[0, c] = base
        cur = base + CW
        for k in fulls:
            seg_off[k, c] = cur
            cur += CW
        for k in range(1, kmax):
            if lo < widths[k] < hi:
                wp = widths[k] - lo
                partials.append((k, wp, cur))
                seg_off[k, c] = cur
                cur += wp
        chunk_meta.append({"base": base, "fulls": len(fulls),
                           "partials": [(wp, off) for _, wp, off in partials]})
        base = cur
    return widths, chunk_meta, seg_off, base, kmax


def _shard_inputs(x, edge_index, edge_attr, u, batch, W1, b1, W2, b2):
    x = np.asarray(x, dtype=np.float32)
    edge_attr = np.asarray(edge_attr, dtype=np.float32)
    u = np.asarray(u, dtype=np.float32)
    batch = np.asarray(batch).astype(np.int64)
    W1 = np.asarray(W1, dtype=np.float32)
    b1 = np.asarray(b1, dtype=np.float32)
    W2 = np.asarray(W2, dtype=np.float32)
    b2 = np.asarray(b2, dtype=np.float32)
    col = np.asarray(edge_index[1]).astype(np.int64)

    deg = np.bincount(col, minlength=N_PAD).astype(np.int64)
    order_nodes = np.argsort(-deg, kind="stable")        # rank -> node id
    rank_of_node = np.empty(N_PAD, dtype=np.int64)
    rank_of_node[order_nodes] = np.arange(N_PAD)

    widths, chunk_meta, seg_off, ct, kmax = _plan_passes(deg, order_nodes)

    # --- edge slab assembly -------------------------------------------------
    r = rank_of_node[col]
    order_e = np.argsort(r, kind="stable")
    rs = r[order_e]
    cnt = np.bincount(rs, minlength=N_PAD)
    starts = np.concatenate([[0], np.cumsum(cnt)])[:-1]
    j = np.arange(N_EDGES, dtype=np.int64) - starts[rs]  # edge index within node
    core_e = rs % NCORES
    pos_e = rs // NCORES
    c_e = pos_e // CW
    dramcol = seg_off[j, c_e] + (pos_e - c_e * CW)
    assert dramcol.min() >= 0

    eap = np.zeros((NCORES, ct, D), dtype=BF16)
    eap[core_e, dramcol] = edge_attr[order_e].astype(BF16)
    eap_all = np.ascontiguousarray(eap.transpose(0, 2, 1))  # [core, 128, ct]

    # --- node-feature relayout ---------------------------------------------
    nodes_by_core = order_nodes.reshape(NPC, NCORES).T      # [core, pos]
    x_pad = np.zeros((N_PAD, D), dtype=np.float32)
    x_pad[:N_NODES] = x
    xt_all = np.ascontiguousarray(
        x_pad[nodes_by_core].transpose(0, 2, 1)).astype(BF16)
    batch_pad = np.concatenate(
        [batch, np.zeros(N_PAD - N_NODES, dtype=np.int64)])
    ug = u[batch_pad]                                       # [N_PAD, DG]
    ugt_all = np.ascontiguousarray(
        ug[nodes_by_core].transpose(0, 2, 1)).astype(BF16)

    consts = {
        "w1x": np.ascontiguousarray(W1[:D]).astype(BF16),          # [128, 256]
        "w1a": np.ascontiguousarray(W1[D:2 * D]).astype(BF16),     # [128, 256]
        "w1u": np.ascontiguousarray(W1[2 * D:]).astype(BF16),      # [16, 256]
        "b1t": np.ascontiguousarray(b1.reshape(2, D).T),           # [128, 2] f32
        "w2a": np.ascontiguousarray(W2[:D]).astype(BF16),          # [128, 128]
        "w2b": np.ascontiguousarray(W2[D:]).astype(BF16),          # [128, 128]
        "b2r": np.ascontiguousarray(b2[None, :]).astype(BF16),     # [1, 128]
        "ones": np.ones((1, NB), dtype=BF16),
        "ident": np.eye(D, dtype=np.float32).astype(BF16),
    }

    in_maps = []
    for c in range(NCORES):
        m = {"eap": eap_all[c], "xt": xt_all[c], "ugt": ugt_all[c]}
        m.update(consts)
        in_maps.append(m)
    return in_maps, chunk_meta, ct, nodes_by_core


def _build_program(chunk_meta, ct):
    import concourse.bacc as bacc
    import concourse.mybir as mybir
    import concourse.tile as tile

    F32 = mybir.dt.float32
    BF = mybir.dt.bfloat16
    Add = mybir.AluOpType.add
    Relu = mybir.ActivationFunctionType.Relu

    nc = bacc.Bacc("TRN2", target_bir_lowering=False, debug=False)

    eap_d = nc.dram_tensor("eap", [D, ct], BF, kind="ExternalInput")
    xt_d = nc.dram_tensor("xt", [D, NPC], BF, kind="ExternalInput")
    ugt_d = nc.dram_tensor("ugt", [DG, NPC], BF, kind="ExternalInput")
    w1x_d = nc.dram_tensor("w1x", [D, H], BF, kind="ExternalInput")
    w1a_d = nc.dram_tensor("w1a", [D, H], BF, kind="ExternalInput")
    w1u_d = nc.dram_tensor("w1u", [DG, H], BF, kind="ExternalInput")
    b1t_d = nc.dram_tensor("b1t", [D, 2], F32, kind="ExternalInput")
    w2a_d = nc.dram_tensor("w2a", [D, D], BF, kind="ExternalInput")
    w2b_d = nc.dram_tensor("w2b", [D, D], BF, kind="ExternalInput")
    b2r_d = nc.dram_tensor("b2r", [1, D], BF, kind="ExternalInput")
    ones_d = nc.dram_tensor("ones", [1, NB], BF, kind="ExternalInput")
    ident_d = nc.dram_tensor("ident", [D, D], BF, kind="ExternalInput")
    out_d = nc.dram_tensor("out", [D, NPC], BF, kind="ExternalOutput")

    with tile.TileContext(nc) as tc, ExitStack() as ctx:
        persist = ctx.enter_context(tc.tile_pool(name="persist", bufs=1))
        agg_pool = ctx.enter_context(tc.tile_pool(name="agg", bufs=1))
        outc_pool = ctx.enter_context(tc.tile_pool(name="outc", bufs=NCHUNK))
        hs_pool = ctx.enter_context(tc.tile_pool(name="hs", bufs=4))
        h_psum = ctx.enter_context(tc.tile_pool(name="hps", bufs=4, space="PSUM"))
        o2_psum = ctx.enter_context(tc.tile_pool(name="o2ps", bufs=2, space="PSUM"))
        wu_psum = ctx.enter_context(tc.tile_pool(name="wups", bufs=1, space="PSUM"))

        def pload(dram, shape, dtype):
            t = persist.tile(shape, dtype, tag=dram.name)
            nc.scalar.dma_start(t[:], dram.ap())
            return t

        w1x_t = pload(w1x_d, [D, H], BF)
        w1a_t = pload(w1a_d, [D, H], BF)
        w1u_t = pload(w1u_d, [DG, H], BF)
        b1t_t = pload(b1t_d, [D, 2], F32)
        w2a_t = pload(w2a_d, [D, D], BF)
        w2b_t = pload(w2b_d, [D, D], BF)
        b2r_t = pload(b2r_d, [1, D], BF)
        ones_t = pload(ones_d, [1, NB], BF)
        ident_t = pload(ident_d, [D, D], BF)
        xt_t = pload(xt_d, [D, NPC], BF)
        ugt_t = pload(ugt_d, [DG, NPC], BF)

        # --- scatter: big HWDGE slab loads + DVE add trees per chunk -------
        # Full passes load 4-at-a-time (one 3.2 MB DMA); a pair-add tree
        # reduces each load with only ONE serialized add onto agg, so the
        # per-chunk dependency chain is ~4x shorter than a naive chain.
        RUN = 4
        slab_pool = ctx.enter_context(tc.tile_pool(name="slab", bufs=5))
        agg_tiles = {}
        for c in range(NCHUNK):
            m = chunk_meta[c]
            eng = nc.vector
            agg = agg_pool.tile([D, CW], BF, tag=f"agg{c}")
            agg_tiles[c] = agg
            nc.sync.dma_start(agg[:], eap_d.ap()[:, m["base"]:m["base"] + CW])
            fb = m["base"] + CW
            i = 0
            nfull = m["fulls"]
            while i < nfull:
                n = min(RUN, nfull - i)
                t = slab_pool.tile([D, RUN * CW], BF, tag="slab")
                nc.sync.dma_start(
                    t[:, 0:n * CW],
                    eap_d.ap()[:, fb + i * CW:fb + (i + n) * CW])
                wu = wu_psum.tile([D, NB], F32, tag="wu")
                nc.tensor.matmul(wu[:], w2a_t[:], t[:, 0:NB], start=True,
                                 stop=True)
                if n == 4:
                    eng.tensor_add(t[:, 0:2 * CW], t[:, 0:2 * CW],
                                   t[:, 2 * CW:4 * CW])
                    eng.tensor_add(t[:, 0:CW], t[:, 0:CW], t[:, CW:2 * CW])
                    eng.tensor_add(agg[:], agg[:], t[:, 0:CW])
                elif n == 3:
                    eng.tensor_add(t[:, 0:CW], t[:, 0:CW], t[:, CW:2 * CW])
                    eng.tensor_add(t[:, 0:CW], t[:, 0:CW], t[:, 2 * CW:3 * CW])
                    eng.tensor_add(agg[:], agg[:], t[:, 0:CW])
                elif n == 2:
                    eng.tensor_add(t[:, 0:CW], t[:, 0:CW], t[:, CW:2 * CW])
                    eng.tensor_add(agg[:], agg[:], t[:, 0:CW])
                else:
                    eng.tensor_add(agg[:], agg[:], t[:, 0:CW])
                i += n
            # partial passes: pack consecutive segs into shared loads
            parts = m["partials"]
            i = 0
            while i < len(parts):
                n = 0
                tot = 0
                while (i + n < len(parts) and tot + parts[i + n][0] <= RUN * CW):
                    tot += parts[i + n][0]
                    n += 1
                n = max(n, 1)
                tot = sum(wp for wp, _ in parts[i:i + n])
                t = slab_pool.tile([D, RUN * CW], BF, tag="slab")
                base_off = parts[i][1]
                nc.sync.dma_start(t[:, 0:tot],
                                  eap_d.ap()[:, base_off:base_off + tot])
                o = 0
                for wp, _ in parts[i:i + n]:
                    eng.tensor_add(agg[:, 0:wp], agg[:, 0:wp],
                                 t[:, o:o + wp])
                    o += wp
                i += n

        # --- MLP over 392-node groups, chunk by chunk ----------------------
        for c in range(NCHUNK):
            agg = agg_tiles[c]
            outc = outc_pool.tile([D, CW], BF, tag="outc")
            for q in range(CW // NB):
                off = q * NB
                gs = c * CW + off
                hs = []
                for ht in range(2):
                    hp = h_psum.tile([D, NB], F32, tag="hp")
                    hsl = slice(ht * D, (ht + 1) * D)
                    nc.tensor.matmul(hp[:], w1x_t[:, hsl], xt_t[:, gs:gs + NB],
                                     start=True, stop=False)
                    nc.tensor.matmul(hp[:], w1a_t[:, hsl], agg[:, off:off + NB],
                                     start=False, stop=False)
                    nc.tensor.matmul(hp[:], w1u_t[:, hsl], ugt_t[:, gs:gs + NB],
                                     start=False, stop=True)
                    hsb = hs_pool.tile([D, NB], BF, tag="hs")
                    nc.scalar.activation(hsb[:], hp[:], Relu,
                                         bias=b1t_t[:, ht:ht + 1])
                    hs.append(hsb)
                o2 = o2_psum.tile([D, NB], F32, tag="o2")
                nc.tensor.matmul(o2[:], w2a_t[:], hs[0][:], start=True, stop=False)
                nc.tensor.matmul(o2[:], w2b_t[:], hs[1][:], start=False, stop=False)
                nc.tensor.matmul(o2[:], ident_t[:], xt_t[:, gs:gs + NB],
                                 start=False, stop=False)
                nc.tensor.matmul(o2[:], b2r_t[:], ones_t[:], start=False, stop=True)
                nc.vector.tensor_copy(outc[:, off:off + NB], o2[:])
                nc.scalar.dma_start(out_d.ap()[:, gs:gs + NB],
                                    outc[:, off:off + NB])

    nc.compile()
    return nc


def kernel(**inputs) -> np.ndarray:
    in_maps, chunk_meta, ct, nodes_by_core = _shard_inputs(
        inputs["x"], inputs["edge_index"], inputs["edge_attr"], inputs["u"],
        inputs["batch"], inputs["W1"], inputs["b1"], inputs["W2"], inputs["b2"],
    )
    nc = _build_program(chunk_meta, ct)

    from concourse.bass_utils import run_bass_kernel_spmd

    res = run_bass_kernel_spmd(nc, in_maps, list(range(NCORES)))
    _PROFILE_RESULTS[0] = res
    full = np.empty((N_PAD, D), dtype=np.float32)
    for c in range(NCORES):
        full[nodes_by_core[c]] = res.results[c]["out"].astype(np.float32).T
    return np.ascontiguousarray(full[:N_NODES])


# revision 15
# speedup vs baseline: 1.2525x; 1.2525x over previous
"""Trainium2 Bass kernel for nn_NodeModel (GNN message passing).

Reference computation:
    agg = segment_sum(edge_attr, edge_index[1], num_segments=N)     # scatter-add
    h   = relu(concat([x, agg, u[batch]], 1) @ W1 + b1)
    out = h @ W2 + b2 + x

Strategy (8 NeuronCores, graph-parallel by destination node):
  - Nodes are sorted by in-degree (descending) and dealt round-robin across
    the 8 cores, so every core sees the same degree profile. Per core the
    12544 owned nodes form the columns of all on-chip tensors.
  - The scatter-add runs as slab adds: the host lays the k-th incoming
    edge row of every node out as "pass k" (a [128, n_k] bf16 slab,
    n_k = #nodes with degree > k; nodes sorted by degree make every pass
    a dense prefix). Pass 0 is DMAed straight into agg; passes 1.. are
    DMAed to a staging tile (HWDGE, full rate) and added into agg by DVE
    tensor_add ops (~0.5 cyc/element bf16). No per-edge compute anywhere.
  - agg is chunked into 8 column ranges with independent add chains so
    chunks pipeline; the MLP consumes chunks as their chains complete.
  - MLP: h1T[h,n] accumulated in PSUM from W1x.T xT + W1a.T aggT + W1u.T ugT;
    ReLU+bias on ScalarE during evacuation; layer 2 in transposed
    orientation with the residual (+x) as an identity matmul and the bias
    as a rank-1 matmul. Output stays transposed [d, n] bf16 on device; the
    host de-transposes and un-permutes.
  - Everything is bf16 on the wire (rel err ~5e-3 incl. bf16 accumulation).
"""

from contextlib import ExitStack

import ml_dtypes
import numpy as np

N_NODES = 100000
N_EDGES = 1600000
D = 128          # node / edge feature dim
DG = 16          # global feature dim
H = 256          # hidden dim
G = 64           # graphs
NCORES = 8

NPC = 12544      # nodes per core
N_PAD = NCORES * NPC
CW = 1568        # agg chunk width (8 chunks per core)
NCHUNK = NPC // CW
NB = 392         # MLP group columns (CW % NB == 0)
MIN_PW = 64      # minimum pass width
PW_ALIGN = 1     # pass width alignment

BF16 = ml_dtypes.bfloat16

_PROFILE_RESULTS = [None]  # stash for test harness introspection


def _plan_passes(deg, order_nodes):
    """Shared-across-cores pass widths and per-chunk DRAM layout."""
    degmat = deg[order_nodes].reshape(NPC, NCORES)      # [pos, core]
    kmax = int(degmat.max())
    ks = np.arange(1, kmax)
    # count per core of nodes with degree > k  -> max over cores
    counts = (degmat[:, :, None] > ks[None, None, :]).sum(axis=0)  # [core, k-1]
    wk = counts.max(axis=0)

    widths = [NPC]
    for k in range(1, kmax):
        w = max(int(wk[k - 1]), MIN_PW)
        w = min(-(-w // PW_ALIGN) * PW_ALIGN, NPC)
        widths.append(w)

    seg_off = np.full((max(kmax, 1), NCHUNK), -1, dtype=np.int64)
    chunk_meta = []
    base = 0
    for c in range(NCHUNK):
        lo, hi = c * CW, (c + 1) * CW
        fulls = [k for k in range(1, kmax) if widths[k] >= hi]
        partials = []
        seg_off[0, c] = base
        cur = base + CW
        for k in fulls:
            seg_off[k, c] = cur
            cur += CW
        for k in range(1, kmax):
            if lo < widths[k] < hi:
                wp = widths[k] - lo
                partials.append((k, wp, cur))
                seg_off[k, c] = cur
                cur += wp
        chunk_meta.append({"base": base, "fulls": len(fulls),
                           "partials": [(wp, off) for _, wp, off in partials]})
        base = cur
    return widths, chunk_meta, seg_off, base, kmax


def _shard_inputs(x, edge_index, edge_attr, u, batch, W1, b1, W2, b2):
    x = np.asarray(x, dtype=np.float32)
    edge_attr = np.asarray(edge_attr, dtype=np.float32)
    u = np.asarray(u, dtype=np.float32)
    batch = np.asarray(batch).astype(np.int64)
    W1 = np.asarray(W1, dtype=np.float32)
    b1 = np.asarray(b1, dtype=np.float32)
    W2 = np.asarray(W2, dtype=np.float32)
    b2 = np.asarray(b2, dtype=np.float32)
    col = np.asarray(edge_index[1]).astype(np.int64)

    deg = np.bincount(col, minlength=N_PAD).astype(np.int64)
    order_nodes = np.argsort(-deg, kind="stable")        # rank -> node id
    rank_of_node = np.empty(N_PAD, dtype=np.int64)
    rank_of_node[order_nodes] = np.arange(N_PAD)

    widths, chunk_meta, seg_off, ct, kmax = _plan_passes(deg, order_nodes)

    # --- edge slab assembly -------------------------------------------------
    r = rank_of_node[col]
    order_e = np.argsort(r, kind="stable")
    rs = r[order_e]
    cnt = np.bincount(rs, minlength=N_PAD)
    starts = np.concatenate([[0], np.cumsum(cnt)])[:-1]
    j = np.arange(N_EDGES, dtype=np.int64) - starts[rs]  # edge index within node
    core_e = rs % NCORES
    pos_e = rs // NCORES
    c_e = pos_e // CW
    dramcol = seg_off[j, c_e] + (pos_e - c_e * CW)
    assert dramcol.min() >= 0

    eap = np.zeros((NCORES, ct, D), dtype=BF16)
    eap[core_e, dramcol] = edge_attr[order_e].astype(BF16)
    eap_all = np.ascontiguousarray(eap.transpose(0, 2, 1))  # [core, 128, ct]

    # --- node-feature relayout ---------------------------------------------
    nodes_by_core = order_nodes.reshape(NPC, NCORES).T      # [core, pos]
    x_pad = np.zeros((N_PAD, D), dtype=np.float32)
    x_pad[:N_NODES] = x
    xt_all = np.ascontiguousarray(
        x_pad[nodes_by_core].transpose(0, 2, 1)).astype(BF16)
    batch_pad = np.concatenate(
        [batch, np.zeros(N_PAD - N_NODES, dtype=np.int64)])
    ug = u[batch_pad]                                       # [N_PAD, DG]
    ugt_all = np.ascontiguousarray(
        ug[nodes_by_core].transpose(0, 2, 1)).astype(BF16)

    consts = {
        "w1x": np.ascontiguousarray(W1[:D]).astype(BF16),          # [128, 256]
        "w1a": np.ascontiguousarray(W1[D:2 * D]).astype(BF16),     # [128, 256]
        "w1u": np.ascontiguousarray(W1[2 * D:]).astype(BF16),      # [16, 256]
        "b1t": np.ascontiguousarray(b1.reshape(2, D).T),           # [128, 2] f32
        "w2a": np.ascontiguousarray(W2[:D]).astype(BF16),          # [128, 128]
        "w2b": np.ascontiguousarray(W2[D:]).astype(BF16),          # [128, 128]
        "b2r": np.ascontiguousarray(b2[None, :]).astype(BF16),     # [1, 128]
        "ones": np.ones((1, NB), dtype=BF16),
        "ident": np.eye(D, dtype=np.float32).astype(BF16),
    }

    in_maps = []
    for c in range(NCORES):
        m = {"eap": eap_all[c], "xt": xt_all[c], "ugt": ugt_all[c]}
        m.update(consts)
        in_maps.append(m)
    return in_maps, chunk_meta, ct, nodes_by_core


def _build_program(chunk_meta, ct):
    import concourse.bacc as bacc
    import concourse.mybir as mybir
    import concourse.tile as tile

    F32 = mybir.dt.float32
    BF = mybir.dt.bfloat16
    Add = mybir.AluOpType.add
    Relu = mybir.ActivationFunctionType.Relu

    nc = bacc.Bacc("TRN2", target_bir_lowering=False, debug=False)

    eap_d = nc.dram_tensor("eap", [D, ct], BF, kind="ExternalInput")
    xt_d = nc.dram_tensor("xt", [D, NPC], BF, kind="ExternalInput")
    ugt_d = nc.dram_tensor("ugt", [DG, NPC], BF, kind="ExternalInput")
    w1x_d = nc.dram_tensor("w1x", [D, H], BF, kind="ExternalInput")
    w1a_d = nc.dram_tensor("w1a", [D, H], BF, kind="ExternalInput")
    w1u_d = nc.dram_tensor("w1u", [DG, H], BF, kind="ExternalInput")
    b1t_d = nc.dram_tensor("b1t", [D, 2], F32, kind="ExternalInput")
    w2a_d = nc.dram_tensor("w2a", [D, D], BF, kind="ExternalInput")
    w2b_d = nc.dram_tensor("w2b", [D, D], BF, kind="ExternalInput")
    b2r_d = nc.dram_tensor("b2r", [1, D], BF, kind="ExternalInput")
    ones_d = nc.dram_tensor("ones", [1, NB], BF, kind="ExternalInput")
    ident_d = nc.dram_tensor("ident", [D, D], BF, kind="ExternalInput")
    out_d = nc.dram_tensor("out", [D, NPC], BF, kind="ExternalOutput")

    with tile.TileContext(nc) as tc, ExitStack() as ctx:
        persist = ctx.enter_context(tc.tile_pool(name="persist", bufs=1))
        agg_pool = ctx.enter_context(tc.tile_pool(name="agg", bufs=1))
        outc_pool = ctx.enter_context(tc.tile_pool(name="outc", bufs=NCHUNK))
        hs_pool = ctx.enter_context(tc.tile_pool(name="hs", bufs=4))
        h_psum = ctx.enter_context(tc.tile_pool(name="hps", bufs=4, space="PSUM"))
        o2_psum = ctx.enter_context(tc.tile_pool(name="o2ps", bufs=2, space="PSUM"))
        wu_psum = ctx.enter_context(tc.tile_pool(name="wups", bufs=1, space="PSUM"))

        def pload(dram, shape, dtype):
            t = persist.tile(shape, dtype, tag=dram.name)
            nc.scalar.dma_start(t[:], dram.ap())
            return t

        w1x_t = pload(w1x_d, [D, H], BF)
        w1a_t = pload(w1a_d, [D, H], BF)
        w1u_t = pload(w1u_d, [DG, H], BF)
        b1t_t = pload(b1t_d, [D, 2], F32)
        w2a_t = pload(w2a_d, [D, D], BF)
        w2b_t = pload(w2b_d, [D, D], BF)
        b2r_t = pload(b2r_d, [1, D], BF)
        ones_t = pload(ones_d, [1, NB], BF)
        ident_t = pload(ident_d, [D, D], BF)
        xt_t = pload(xt_d, [D, NPC], BF)
        ugt_t = pload(ugt_d, [DG, NPC], BF)

        # --- scatter: big HWDGE slab loads + DVE add trees per chunk -------
        # Full passes load 4-at-a-time (one 3.2 MB DMA); a pair-add tree
        # reduces each load with only ONE serialized add onto agg, so the
        # per-chunk dependency chain is ~4x shorter than a naive chain.
        RUN = 4
        slab_pool = ctx.enter_context(tc.tile_pool(name="slab", bufs=5))
        tmp_pool = ctx.enter_context(tc.tile_pool(name="tmp", bufs=6))
        agg_tiles = {}
        for c in range(NCHUNK):
            m = chunk_meta[c]
            eng = nc.vector
            agg = agg_pool.tile([D, CW], BF, tag=f"agg{c}")
            agg_tiles[c] = agg
            nc.sync.dma_start(agg[:], eap_d.ap()[:, m["base"]:m["base"] + CW])
            fb = m["base"] + CW
            i = 0
            nfull = m["fulls"]
            while i < nfull:
                n = min(RUN, nfull - i)
                t = slab_pool.tile([D, RUN * CW], BF, tag="slab")
                nc.sync.dma_start(
                    t[:, 0:n * CW],
                    eap_d.ap()[:, fb + i * CW:fb + (i + n) * CW])
                wu = wu_psum.tile([D, NB], F32, tag="wu")
                nc.tensor.matmul(wu[:], w2a_t[:], t[:, 0:NB], start=True,
                                 stop=True)
                if n == 4:
                    a = tmp_pool.tile([D, CW], BF, tag="tmp")
                    b = tmp_pool.tile([D, CW], BF, tag="tmp")
                    eng.tensor_add(a[:], t[:, 0:CW], t[:, CW:2 * CW])
                    eng.tensor_add(b[:], t[:, 2 * CW:3 * CW],
                                 t[:, 3 * CW:4 * CW])
                    eng.tensor_add(a[:], a[:], b[:])
                    eng.tensor_add(agg[:], agg[:], a[:])
                elif n == 3:
                    a = tmp_pool.tile([D, CW], BF, tag="tmp")
                    eng.tensor_add(a[:], t[:, 0:CW], t[:, CW:2 * CW])
                    eng.tensor_add(a[:], a[:], t[:, 2 * CW:3 * CW])
                    eng.tensor_add(agg[:], agg[:], a[:])
                elif n == 2:
                    a = tmp_pool.tile([D, CW], BF, tag="tmp")
                    eng.tensor_add(a[:], t[:, 0:CW], t[:, CW:2 * CW])
                    eng.tensor_add(agg[:], agg[:], a[:])
                else:
                    eng.tensor_add(agg[:], agg[:], t[:, 0:CW])
                i += n
            # partial passes: pack consecutive segs into shared loads
            parts = m["partials"]
            i = 0
            while i < len(parts):
                n = 0
                tot = 0
                while (i + n < len(parts) and tot + parts[i + n][0] <= RUN * CW):
                    tot += parts[i + n][0]
                    n += 1
                n = max(n, 1)
                tot = sum(wp for wp, _ in parts[i:i + n])
                t = slab_pool.tile([D, RUN * CW], BF, tag="slab")
                base_off = parts[i][1]
                nc.sync.dma_start(t[:, 0:tot],
                                  eap_d.ap()[:, base_off:base_off + tot])
                o = 0
                for wp, _ in parts[i:i + n]:
                    eng.tensor_add(agg[:, 0:wp], agg[:, 0:wp],
                                 t[:, o:o + wp])
                    o += wp
                i += n

        # --- MLP over 392-node groups, chunk by chunk ----------------------
        for c in range(NCHUNK):
            agg = agg_tiles[c]
            outc = outc_pool.tile([D, CW], BF, tag="outc")
            for q in range(CW // NB):
                off = q * NB
                gs = c * CW + off
                hs = []
                for ht in range(2):
                    hp = h_psum.tile([D, NB], F32, tag="hp")
                    hsl = slice(ht * D, (ht + 1) * D)
                    nc.tensor.matmul(hp[:], w1x_t[:, hsl], xt_t[:, gs:gs + NB],
                                     start=True, stop=False)
                    nc.tensor.matmul(hp[:], w1a_t[:, hsl], agg[:, off:off + NB],
                                     start=False, stop=False)
                    nc.tensor.matmul(hp[:], w1u_t[:, hsl], ugt_t[:, gs:gs + NB],
                                     start=False, stop=True)
                    hsb = hs_pool.tile([D, NB], BF, tag="hs")
                    nc.scalar.activation(hsb[:], hp[:], Relu,
                                         bias=b1t_t[:, ht:ht + 1])
                    hs.append(hsb)
                o2 = o2_psum.tile([D, NB], F32, tag="o2")
                nc.tensor.matmul(o2[:], w2a_t[:], hs[0][:], start=True, stop=False)
                nc.tensor.matmul(o2[:], w2b_t[:], hs[1][:], start=False, stop=False)
                nc.tensor.matmul(o2[:], ident_t[:], xt_t[:, gs:gs + NB],
                                 start=False, stop=False)
                nc.tensor.matmul(o2[:], b2r_t[:], ones_t[:], start=False, stop=True)
                nc.scalar.copy(outc[:, off:off + NB], o2[:])
                nc.scalar.dma_start(out_d.ap()[:, gs:gs + NB],
                                    outc[:, off:off + NB])

    nc.compile()
    return nc


def kernel(**inputs) -> np.ndarray:
    in_maps, chunk_meta, ct, nodes_by_core = _shard_inputs(
        inputs["x"], inputs["edge_index"], inputs["edge_attr"], inputs["u"],
        inputs["batch"], inputs["W1"], inputs["b1"], inputs["W2"], inputs["b2"],
    )
    nc = _build_program(chunk_meta, ct)

    from concourse.bass_utils import run_bass_kernel_spmd

    res = run_bass_kernel_spmd(nc, in_maps, list(range(NCORES)))
    _PROFILE_RESULTS[0] = res
    full = np.empty((N_PAD, D), dtype=np.float32)
    for c in range(NCORES):
        full[nodes_by_core[c]] = res.results[c]["out"].astype(np.float32).T
    return np.ascontiguousarray(full[:N_NODES])
